# revision 1
# baseline (speedup 1.0000x reference)
"""DeepSeek MLA decode-step kernel for 8 Trainium2 NeuronCores.

Strategy
--------
- Data-parallel over batch: B=8 -> one batch element per core, no collectives.
- MLA weight absorption: the latent cache is never decompressed. Scores are
  computed against an augmented latent z = [c_kv(512) ; k_rope per head(16*64)]
  (1536 dims) with a block-sparse augmented query Q_aug built on-chip:
      scores[(h,t), s] = q_abs[h,t,:512] . c_kv[s] + q_rope[h,t] . k_rope[s,h]
  where q_abs = q_content @ w_kv_up[:, h, :C].T  (absorbed).
  The attention output stays in latent space (o_lat = attn @ c_kv), is pushed
  through w_kv_up per head, then through w_out.
- Host prep only reshapes/transposes/concats inputs (no FLOPs besides the tiny
  4x32 rope cos/sin table): zT = [c_kv.T ; k_rope.T] per batch, x.T, fused
  input-projection weight, per-head-transposed content slice of w_kv_up.
- Matmuls run as float32r (full PE rate at free-dim >= 256), fp32 accumulate.

Per-core HBM traffic ~85 MB, PE ~100 us worth of work -> memory-bound.
"""

import numpy as np

import concourse.bass as bass
import concourse.mybir as mybir
import concourse.tile as tile
from concourse import bacc
from concourse import bass_utils
from concourse.masks import make_identity

# Problem dims (hardcoded per contract)
B, T, S0 = 8, 4, 4096
D_MODEL, H, D, R, RANK = 2048, 16, 128, 64, 512
C = D - R                  # 64 content dims per head
S = S0 + T                 # 4100 total positions
Z = RANK + H * R           # 1536 augmented latent dim
ZKT = Z // 128             # 12 k-tiles over latent dim
PROJ_N = H * C + H * R + RANK + H * R   # 3584 fused projection cols
SCHUNK = 512               # score-chunk width over cache positions
NCHUNKS = S0 // SCHUNK     # 8
F32 = mybir.dt.float32
F32R = mybir.dt.float32r

_BUILT = None  # (nc,) cached across calls in one process
_LAST_RESULTS = None  # BassKernelResults of the most recent run (for test harness)


BF16 = mybir.dt.bfloat16


def build_bass():
    nc = bacc.Bacc("TRN2", target_bir_lowering=False, debug=False, num_devices=8)

    # ---- per-core DRAM I/O ----
    d_xT = nc.dram_tensor("xT", [D_MODEL, T], F32, kind="ExternalInput").ap()
    d_win = nc.dram_tensor("w_in", [D_MODEL, PROJ_N], F32, kind="ExternalInput").ap()
    d_wup = nc.dram_tensor("w_up", [RANK, H * D], F32, kind="ExternalInput").ap()
    d_wuptc = nc.dram_tensor("w_up_tc", [C, H * RANK], F32, kind="ExternalInput").ap()
    d_wout = nc.dram_tensor("w_out", [H * D, D_MODEL], F32, kind="ExternalInput").ap()
    d_zt = nc.dram_tensor("z_t", [Z, S0], F32, kind="ExternalInput").ap()
    d_ckv = nc.dram_tensor("c_kv", [S0, RANK], F32, kind="ExternalInput").ap()
    d_ctab = nc.dram_tensor("ctab", [T, H * R], F32, kind="ExternalInput").ap()
    d_stab = nc.dram_tensor("stab", [T, H * R], F32, kind="ExternalInput").ap()
    d_out = nc.dram_tensor("out", [T, D_MODEL], F32, kind="ExternalOutput").ap()

    with tile.TileContext(nc) as tc:
        with (
            tc.tile_pool(name="singles", bufs=1) as singles,
            tc.tile_pool(name="small", bufs=2) as small,
            tc.tile_pool(name="win_pool", bufs=2) as win_pool,
            tc.tile_pool(name="zt_pool", bufs=2) as zt_pool,
            tc.tile_pool(name="ckv_pool", bufs=3) as ckv_pool,
            tc.tile_pool(name="wout_pool", bufs=2) as wout_pool,
            tc.tile_pool(name="tp_psum", bufs=2, space="PSUM") as tp_psum,
        ):
            # ---------- resident SBUF loads ----------
            identity = singles.tile([128, 128], F32)
            make_identity(nc, identity)

            # gpsimd (SWDGE) DMAs cast f32->bf16 in flight: HBM traffic stays
            # f32, SBUF gets bf16, and no compute engine spends cycles casting.
            xT_bf = singles.tile([128, D_MODEL // 128, T], BF16)
            nc.gpsimd.dma_start(
                out=xT_bf, in_=d_xT.rearrange("(kt p) t -> p kt t", p=128)
            )
            wup_sb = singles.tile([128, RANK // 128, H * D], F32)  # [128,4,2048]
            nc.sync.dma_start(
                out=wup_sb, in_=d_wup.rearrange("(kt p) n -> p kt n", p=128)
            )
            wuptc_sb = singles.tile([C, H * RANK], F32)  # [64, 8192]
            nc.sync.dma_start(out=wuptc_sb, in_=d_wuptc)
            ctab_sb = singles.tile([T, H * R], F32)
            nc.sync.dma_start(out=ctab_sb, in_=d_ctab)
            stab_sb = singles.tile([T, H * R], F32)
            nc.sync.dma_start(out=stab_sb, in_=d_stab)

            # ---------- stage 1: fused input projection  proj = x @ W_in ----------
            # proj rows live in partitions 0..3; partitions 4..127 zeroed so the
            # later PE transposes can use full-partition inputs.
            proj_sb = singles.tile([128, PROJ_N], F32)
            nc.vector.memset(proj_sb, 0.0)
            with tc.tile_pool(name="proj_psum", bufs=2, space="PSUM") as proj_psum:
                for ci in range(PROJ_N // 512):  # 7 chunks
                    win_bf = win_pool.tile([128, D_MODEL // 128, 512], BF16,
                                           tag="win_bf")
                    nc.gpsimd.dma_start(
                        out=win_bf,
                        in_=d_win.rearrange("(kt p) n -> p kt n", p=128)[
                            :, :, ci * 512 : (ci + 1) * 512
                        ],
                    )
                    ps = proj_psum.tile([T, 512], F32)
                    for kt in range(D_MODEL // 128):  # 16
                        nc.tensor.matmul(
                            ps,
                            xT_bf[:, kt, :],
                            win_bf[:, kt, :],
                            start=(kt == 0),
                            stop=(kt == D_MODEL // 128 - 1),
                        )
                    nc.scalar.activation(
                        out=proj_sb[0:T, ci * 512 : (ci + 1) * 512],
                        in_=ps,
                        func=mybir.ActivationFunctionType.Copy,
                    )

            # views into proj
            q_content = proj_sb[0:T, 0 : H * C]                     # [4,1024]
            q_rope_raw = proj_sb[0:T, H * C : H * C + H * R]        # [4,1024]
            ckv_new = proj_sb[0:T, 2048 : 2048 + RANK]              # [4,512]
            k_rope_raw = proj_sb[0:T, 2560 : 2560 + H * R]          # [4,1024]

            # ---------- stage 2: rope rotation for q_rope and k_rope_new ----------
            rot_q = singles.tile([128, H * R], F32)
            rot_k = singles.tile([128, H * R], F32)
            nc.vector.memset(rot_q, 0.0)
            nc.vector.memset(rot_k, 0.0)
            tmpA = small.tile([T, H * R], F32, tag="ropetmp")
            tmpB = small.tile([T, H * R], F32, tag="ropetmp")
            _rope(nc, rot_q, q_rope_raw, tmpA, tmpB, ctab_sb, stab_sb)
            tmpA2 = small.tile([T, H * R], F32, tag="ropetmp")
            tmpB2 = small.tile([T, H * R], F32, tag="ropetmp")
            _rope(nc, rot_k, k_rope_raw, tmpA2, tmpB2, ctab_sb, stab_sb)

            # ---------- stages 3-5: transposed new-token tensors -----------------
            # Q_augT [1536, 64]: col (h,t) = 4h+t; rows 0..511 = q_abs,
            # rows 512+64h.. = q_rope[h].  z_newT [1536, 4] likewise holds the
            # new tokens' latent (c_kv_new ; k_rope_new).
            # PE transposes evict partition-aligned: transpose j of a [4, 1024]
            # tensor yields dims (128j..128j+127) on partitions 0..127, which is
            # heads (2j, 2j+1) for 64-wide per-head blocks.
            qaugT = singles.tile([128, ZKT, 64], F32)
            znewT = singles.tile([128, ZKT, T], F32)
            nc.vector.memset(qaugT, 0.0)

            # q_contentT per head at base partition 0: even heads come out of the
            # transpose on partitions 0..63 (DVE copy); odd heads land on
            # partitions 64..127 and are moved down with an SBUF->SBUF DMA.
            qcT = singles.tile([C, H, T], F32)   # [64,16,4]
            qcT2 = singles.tile([128, H * C // 128, T], F32)  # [128,8,4]
            for j in range(H * C // 128):  # q_content transposes
                pt = tp_psum.tile([128, 128], F32, tag="tp")
                nc.tensor.transpose(
                    pt, proj_sb[:, j * 128 : (j + 1) * 128], identity
                )
                nc.vector.tensor_copy(out=qcT2[:, j, :], in_=pt[:, 0:T])
                nc.vector.tensor_copy(out=qcT[:, 2 * j, :], in_=pt[0:C, 0:T])
            for j in range(H * C // 128):
                nc.sync.dma_start(
                    out=qcT[:, 2 * j + 1, :], in_=qcT2[C : 2 * C, j, :]
                )
            for j in range(RANK // 128):  # c_kv_new transposes -> znewT rows 0..511
                pt = tp_psum.tile([128, 128], F32, tag="tp")
                nc.tensor.transpose(
                    pt, proj_sb[:, 2048 + j * 128 : 2048 + (j + 1) * 128], identity
                )
                nc.vector.tensor_copy(out=znewT[:, j, :], in_=pt[:, 0:T])
            for j in range(H * R // 128):  # q_rope transposes -> qaugT rope rows
                pt = tp_psum.tile([128, 128], F32, tag="tp")
                nc.tensor.transpose(pt, rot_q[:, j * 128 : (j + 1) * 128], identity)
                nc.vector.tensor_copy(
                    out=qaugT[0:64, 4 + j, 8 * j : 8 * j + 4], in_=pt[0:64, 0:T]
                )
                nc.vector.tensor_copy(
                    out=qaugT[64:128, 4 + j, 8 * j + 4 : 8 * j + 8],
                    in_=pt[64:128, 0:T],
                )
            for j in range(H * R // 128):  # k_rope_new transposes -> znewT rope rows
                pt = tp_psum.tile([128, 128], F32, tag="tp")
                nc.tensor.transpose(pt, rot_k[:, j * 128 : (j + 1) * 128], identity)
                nc.vector.tensor_copy(out=znewT[:, 4 + j, :], in_=pt[:, 0:T])

            # absorbed queries: q_abs[h] = q_content[h] @ w_up_tc[h]  -> rows 0..511
            with tc.tile_pool(name="qabs_psum", bufs=2, space="PSUM") as qabs_psum:
                for rt in range(RANK // 128):  # 4 r-tiles
                    ps = qabs_psum.tile([128, 64], F32)
                    for h in range(H):
                        nc.tensor.matmul(
                            ps[:, 4 * h : 4 * h + 4],
                            wuptc_sb[
                                :, h * RANK + rt * 128 : h * RANK + (rt + 1) * 128
                            ],
                            qcT[:, h, :],
                            start=True,
                            stop=True,
                        )
                    nc.vector.tensor_copy(out=qaugT[:, rt, :], in_=ps)

            # bf16 copies of the score-matmul stationary operands
            qaugT_bf = singles.tile([128, ZKT, 64], BF16)
            nc.vector.tensor_copy(out=qaugT_bf, in_=qaugT)
            znewT_bf = singles.tile([128, ZKT, T], BF16)
            nc.vector.tensor_copy(out=znewT_bf, in_=znewT)

            identity_bf = singles.tile([128, 128], BF16)
            nc.vector.tensor_copy(out=identity_bf, in_=identity)

            # ---------- stage 6: scores + exp + row-sums ------------------------
            # exp values held in bf16 (they feed bf16 matmuls anyway)
            exp_sb = singles.tile([64, S], BF16)
            sums = singles.tile([64, NCHUNKS + 1], F32)
            with tc.tile_pool(name="sc_psum", bufs=2, space="PSUM") as sc_psum:
                for ci in range(NCHUNKS):  # 8 x 512 cache positions
                    zt_bf = zt_pool.tile([128, ZKT, SCHUNK], BF16, tag="zt_bf")
                    nc.gpsimd.dma_start(
                        out=zt_bf,
                        in_=d_zt.rearrange("(kt p) s -> p kt s", p=128)[
                            :, :, ci * SCHUNK : (ci + 1) * SCHUNK
                        ],
                    )
                    ps = sc_psum.tile([64, SCHUNK], F32, tag="sc")
                    for kt in range(ZKT):
                        nc.tensor.matmul(
                            ps,
                            qaugT_bf[:, kt, :],
                            zt_bf[:, kt, :],
                            start=(kt == 0),
                            stop=(kt == ZKT - 1),
                        )
                    nc.scalar.activation(
                        out=exp_sb[:, ci * SCHUNK : (ci + 1) * SCHUNK],
                        in_=ps,
                        func=mybir.ActivationFunctionType.Exp,
                        accum_out=sums[:, ci : ci + 1],
                    )
                # new-token chunk (4 cols)
                ps9 = sc_psum.tile([64, T], F32, tag="sc9")
                for kt in range(ZKT):
                    nc.tensor.matmul(
                        ps9,
                        qaugT_bf[:, kt, :],
                        znewT_bf[:, kt, :],
                        start=(kt == 0),
                        stop=(kt == ZKT - 1),
                    )
                nc.scalar.activation(
                    out=exp_sb[:, S0:S],
                    in_=ps9,
                    func=mybir.ActivationFunctionType.Exp,
                    accum_out=sums[:, NCHUNKS : NCHUNKS + 1],
                )

            # ---------- stage 7: softmax normalize ------------------------------
            total = singles.tile([64, 1], F32)
            recip = singles.tile([64, 1], F32)
            nc.vector.reduce_sum(out=total, in_=sums, axis=mybir.AxisListType.X)
            nc.vector.reciprocal(out=recip, in_=total)
            nc.vector.tensor_scalar_mul(exp_sb, exp_sb, recip)  # bf16 *= f32 scalar

            # ---------- stage 8: transpose attn -> attnT [4100, 64] -------------
            attnT = singles.tile([128, S0 // 128 + 1, 64], BF16)  # [128,33,64]
            for st in range(S0 // 128):  # 32
                pt = tp_psum.tile([128, 128], BF16, tag="tp")
                nc.tensor.transpose(
                    pt[:, 0:64], exp_sb[:, st * 128 : (st + 1) * 128],
                    identity_bf[0:64, 0:64],
                )
                nc.vector.tensor_copy(out=attnT[:, st, :], in_=pt[:, 0:64])
            ptn = tp_psum.tile([128, 128], BF16, tag="tp")
            nc.tensor.transpose(
                ptn[0:T, 0:64], exp_sb[:, S0:S], identity_bf[0:64, 0:64]
            )
            nc.vector.tensor_copy(out=attnT[0:T, S0 // 128, :], in_=ptn[0:T, 0:64])
            ckv_new_bf = singles.tile([T, RANK], BF16)
            nc.vector.tensor_copy(out=ckv_new_bf, in_=ckv_new)

            # ---------- stage 9: o_lat = attn @ c_kv  [64, 512] ------------------
            o_lat = singles.tile([64, RANK], F32)
            with tc.tile_pool(name="olat_psum", bufs=1, space="PSUM") as olat_psum:
                pso = olat_psum.tile([64, RANK], F32)
                for kg in range(S0 // 512):  # 8 casting DMAs of 4 k-tiles each
                    ckv_bf = ckv_pool.tile([128, 4, RANK], BF16, tag="ckv_bf")
                    nc.gpsimd.dma_start(
                        out=ckv_bf,
                        in_=d_ckv.rearrange("(kg kt p) r -> kg p kt r", kt=4, p=128)[
                            kg
                        ],
                    )
                    for kt in range(4):
                        nc.tensor.matmul(
                            pso,
                            attnT[:, kg * 4 + kt, :],
                            ckv_bf[:, kt, :],
                            start=(kg == 0 and kt == 0),
                            stop=False,
                        )
                nc.tensor.matmul(
                    pso,
                    attnT[0:T, S0 // 128, :],
                    ckv_new_bf,
                    start=False,
                    stop=True,
                )
                nc.vector.tensor_copy(out=o_lat, in_=pso)

            # o_latT [512, 64]
            olatT = singles.tile([128, RANK // 128, 64], F32)
            for rt in range(RANK // 128):
                pt = tp_psum.tile([128, 128], F32, tag="tp")
                nc.tensor.transpose(
                    pt[:, 0:64], o_lat[:, rt * 128 : (rt + 1) * 128],
                    identity[0:64, 0:64],
                )
                nc.vector.tensor_copy(out=olatT[:, rt, :], in_=pt[:, 0:64])

            # ---------- stage 10: attn_outT [2048, 4] = blockdiag(w_up^T) @ o_latT
            aoT = singles.tile([128, H, T], BF16)
            with tc.tile_pool(name="ao_psum", bufs=2, space="PSUM") as ao_psum:
                for h in range(H):
                    ps = ao_psum.tile([128, T], F32)
                    for rt in range(RANK // 128):
                        nc.tensor.matmul(
                            ps,
                            wup_sb[:, rt, h * D : (h + 1) * D],
                            olatT[:, rt, 4 * h : 4 * h + 4],
                            start=(rt == 0),
                            stop=(rt == RANK // 128 - 1),
                        )
                    nc.vector.tensor_copy(out=aoT[:, h, :], in_=ps)

            # ---------- stage 11: out = attn_out @ w_out [4, 2048] ---------------
            with tc.tile_pool(name="out_psum", bufs=1, space="PSUM") as out_psum:
                pouts = [
                    out_psum.tile([T, 512], F32, name=f"pout{i}", tag=f"pout{i}")
                    for i in range(4)
                ]
                for kt in range(H * D // 128):  # 16
                    wout_bf = wout_pool.tile([128, D_MODEL], BF16, tag="wout_bf")
                    nc.gpsimd.dma_start(
                        out=wout_bf, in_=d_wout[kt * 128 : (kt + 1) * 128, :]
                    )
                    for ci in range(4):
                        # D == 128, so attn_outT k-tile kt is exactly head kt
                        nc.tensor.matmul(
                            pouts[ci],
                            aoT[:, kt, :],
                            wout_bf[:, ci * 512 : (ci + 1) * 512],
                            start=(kt == 0),
                            stop=(kt == H * D // 128 - 1),
                        )
                for ci in range(4):
                    # stage through (now-dead) proj_sb columns; DMA can't read PSUM
                    nc.vector.tensor_copy(
                        out=proj_sb[0:T, ci * 512 : (ci + 1) * 512], in_=pouts[ci]
                    )
                    nc.sync.dma_start(
                        out=d_out[:, ci * 512 : (ci + 1) * 512],
                        in_=proj_sb[0:T, ci * 512 : (ci + 1) * 512],
                    )

    nc.compile()
    return nc


def _rope(nc, rot, raw, tmpA, tmpB, ctab_sb, stab_sb):
    """rot[t, h*64+j]      = raw1*cos - raw2*sin   (j in [0,32))
       rot[t, h*64+32+j]   = raw1*sin + raw2*cos
    using A = raw*ctab, B = raw*stab with half-replicated cos/sin tables."""
    nc.vector.tensor_mul(tmpA, raw, ctab_sb)
    nc.vector.tensor_mul(tmpB, raw, stab_sb)
    a3 = tmpA.rearrange("p (h r) -> p h r", h=H)
    b3 = tmpB.rearrange("p (h r) -> p h r", h=H)
    r3 = rot[0:T, :].rearrange("p (h r) -> p h r", h=H)
    nc.vector.tensor_sub(r3[:, :, 0:32], a3[:, :, 0:32], b3[:, :, 32:64])
    nc.vector.tensor_add(r3[:, :, 32:64], b3[:, :, 0:32], a3[:, :, 32:64])


def _prep_shared(w_q, w_q_rope, w_kv_down, w_kv_up, w_k_rope, w_out):
    scale = np.float32(1.0 / np.sqrt(D))
    w_in = np.ascontiguousarray(
        np.concatenate([w_q * scale, w_q_rope * scale, w_kv_down, w_k_rope], axis=1)
    )
    w_up_tc = np.ascontiguousarray(
        w_kv_up.reshape(RANK, H, D)[:, :, :C].transpose(2, 1, 0)
    ).reshape(C, H * RANK)
    pos = S0 + np.arange(T)
    invf = 1.0 / (10000.0 ** (np.arange(R // 2) / (R // 2)))
    f = (pos[:, None] * invf[None, :]).astype(np.float32)
    cos32, sin32 = np.cos(f).astype(np.float32), np.sin(f).astype(np.float32)
    ctab = np.ascontiguousarray(
        np.tile(np.concatenate([cos32, cos32], 1)[:, None, :], (1, H, 1))
    ).reshape(T, H * R)
    stab = np.ascontiguousarray(
        np.tile(np.concatenate([sin32, sin32], 1)[:, None, :], (1, H, 1))
    ).reshape(T, H * R)
    return w_in, np.ascontiguousarray(w_kv_up), w_up_tc, np.ascontiguousarray(w_out), ctab, stab


def _make_in_maps(inputs):
    x = np.asarray(inputs["x"], np.float32)
    cache_c_kv = np.asarray(inputs["cache_c_kv"], np.float32)
    cache_k_rope = np.asarray(inputs["cache_k_rope"], np.float32)
    w_in, w_up, w_up_tc, w_out_c, ctab, stab = _prep_shared(
        np.asarray(inputs["w_q"], np.float32),
        np.asarray(inputs["w_q_rope"], np.float32),
        np.asarray(inputs["w_kv_down"], np.float32),
        np.asarray(inputs["w_kv_up"], np.float32),
        np.asarray(inputs["w_k_rope"], np.float32),
        np.asarray(inputs["w_out"], np.float32))

    in_maps = []
    for b in range(B):
        zt = np.ascontiguousarray(
            np.concatenate(
                [cache_c_kv[b].T,
                 cache_k_rope[b].transpose(1, 2, 0).reshape(H * R, S0)],
                axis=0,
            )
        )
        in_maps.append({
            "xT": np.ascontiguousarray(x[b].T),
            "w_in": w_in,
            "w_up": w_up,
            "w_up_tc": w_up_tc,
            "w_out": w_out_c,
            "z_t": zt,
            "c_kv": np.ascontiguousarray(cache_c_kv[b]),
            "ctab": ctab,
            "stab": stab,
        })
    return in_maps


def kernel(x, cache_c_kv, cache_k_rope, w_q, w_q_rope, w_kv_down, w_kv_up,
           w_k_rope, w_out):
    global _BUILT, _LAST_RESULTS
    if _BUILT is None:
        _BUILT = build_bass()
    nc = _BUILT

    in_maps = _make_in_maps(dict(
        x=x, cache_c_kv=cache_c_kv, cache_k_rope=cache_k_rope, w_q=w_q,
        w_q_rope=w_q_rope, w_kv_down=w_kv_down, w_kv_up=w_kv_up,
        w_k_rope=w_k_rope, w_out=w_out))

    res = bass_utils.run_bass_kernel_spmd(nc, in_maps, core_ids=list(range(B)))
    _LAST_RESULTS = res
    return np.stack([res.results[b]["out"] for b in range(B)]).astype(np.float32)



# revision 3
# speedup vs baseline: 1.9708x; 1.9708x over previous
"""DeepSeek MLA decode-step kernel for 8 Trainium2 NeuronCores.

Strategy
--------
- Data-parallel over batch: B=8 -> one batch element per core, no collectives.
- MLA weight absorption: the latent cache is never decompressed. Scores are
  computed against an augmented latent z = [c_kv(512) ; k_rope per head(16*64)]
  (1536 dims) with a block-sparse augmented query Q_aug built on-chip:
      scores[(h,t), s] = q_abs[h,t,:512] . c_kv[s] + q_rope[h,t] . k_rope[s,h]
  where q_abs = q_content @ w_kv_up[:, h, :C].T  (absorbed).
  The attention output stays in latent space (o_lat = attn @ c_kv), is pushed
  through w_kv_up per head, then through w_out.
- All large operands are cast to bf16 on the host and shipped pre-chunked so
  every DMA lands contiguous per partition on hardware DGE queues (no in-
  flight dtype-casting SWDGE, which caps at ~12 B/ns/engine).
- Per-core HBM traffic ~43 MB -> memory-bound target ~130-150 us.
"""

import numpy as np
import ml_dtypes

import concourse.bass as bass
import concourse.mybir as mybir
import concourse.tile as tile
from concourse import bacc
from concourse import bass_utils
from concourse.masks import make_identity

# Problem dims (hardcoded per contract)
B, T, S0 = 8, 4, 4096
D_MODEL, H, D, R, RANK = 2048, 16, 128, 64, 512
C = D - R                  # 64 content dims per head
S = S0 + T                 # 4100 total positions
Z = RANK + H * R           # 1536 augmented latent dim
ZKT = Z // 128             # 12 k-tiles over latent dim
PROJ_N = H * C + H * R + RANK + H * R   # 3584 fused projection cols
SCHUNK = 512               # score-chunk width over cache positions
NCHUNKS = S0 // SCHUNK     # 8
F32 = mybir.dt.float32
BF16 = mybir.dt.bfloat16
NPBF = ml_dtypes.bfloat16

_BUILT = None  # (nc,) cached across calls in one process
_LAST_RESULTS = None  # BassKernelResults of the most recent run (for test harness)


def build_bass():
    nc = bacc.Bacc("TRN2", target_bir_lowering=False, debug=False, num_devices=8)

    # ---- per-core DRAM I/O (bf16, host-prechunked contiguous layouts) ----
    d_xT = nc.dram_tensor("xT", [128, D_MODEL // 128, T], BF16,
                          kind="ExternalInput").ap()
    d_win = nc.dram_tensor("w_in", [(PROJ_N // 512) * 128, D_MODEL // 128, 512],
                           BF16, kind="ExternalInput").ap()
    d_wup = nc.dram_tensor("w_up", [128, RANK // 128, H * D], BF16,
                           kind="ExternalInput").ap()
    d_wuptc = nc.dram_tensor("w_up_tc", [C, H * RANK], BF16,
                             kind="ExternalInput").ap()
    d_wout = nc.dram_tensor("w_out", [H * D, D_MODEL], BF16,
                            kind="ExternalInput").ap()
    d_zt = nc.dram_tensor("z_t", [NCHUNKS * 128, ZKT, SCHUNK], BF16,
                          kind="ExternalInput").ap()
    d_ckv = nc.dram_tensor("c_kv", [NCHUNKS * 128, S0 // NCHUNKS // 128, RANK],
                           BF16, kind="ExternalInput").ap()
    d_ctab = nc.dram_tensor("ctab", [T, H * R], F32, kind="ExternalInput").ap()
    d_stab = nc.dram_tensor("stab", [T, H * R], F32, kind="ExternalInput").ap()
    d_out = nc.dram_tensor("out", [T, D_MODEL], F32, kind="ExternalOutput").ap()

    with tile.TileContext(nc) as tc:
        with (
            tc.tile_pool(name="singles", bufs=1) as singles,
            tc.tile_pool(name="small", bufs=2) as small,
            tc.tile_pool(name="win_pool", bufs=2) as win_pool,
            tc.tile_pool(name="zt_pool", bufs=3) as zt_pool,
            tc.tile_pool(name="ckv_pool", bufs=3) as ckv_pool,
            tc.tile_pool(name="wout_pool", bufs=3) as wout_pool,
            tc.tile_pool(name="tp_psum", bufs=2, space="PSUM") as tp_psum,
        ):
            # ---------- resident SBUF loads ----------
            identity = singles.tile([128, 128], F32)
            make_identity(nc, identity)
            id_bf = singles.tile([128, 128], BF16)
            nc.vector.tensor_copy(out=id_bf, in_=identity)

            xT_bf = singles.tile([128, D_MODEL // 128, T], BF16)
            nc.sync.dma_start(out=xT_bf, in_=d_xT)
            wup_sb = singles.tile([128, RANK // 128, H * D], BF16)
            nc.sync.dma_start(out=wup_sb, in_=d_wup)
            wuptc_sb = singles.tile([C, H * RANK], BF16)  # [64, 8192]
            nc.sync.dma_start(out=wuptc_sb, in_=d_wuptc)
            ctab_sb = singles.tile([T, H * R], F32)
            nc.sync.dma_start(out=ctab_sb, in_=d_ctab)
            stab_sb = singles.tile([T, H * R], F32)
            nc.sync.dma_start(out=stab_sb, in_=d_stab)

            # ---------- stage 1: fused input projection  proj = x @ W_in ----------
            proj_sb = singles.tile([T, PROJ_N], F32)
            with tc.tile_pool(name="proj_psum", bufs=2, space="PSUM") as proj_psum:
                for ci in range(PROJ_N // 512):  # 7 chunks
                    win_bf = win_pool.tile([128, D_MODEL // 128, 512], BF16,
                                           tag="win_bf")
                    nc.sync.dma_start(
                        out=win_bf, in_=d_win[ci * 128:(ci + 1) * 128]
                    )
                    ps = proj_psum.tile([T, 512], F32)
                    for kt in range(D_MODEL // 128):  # 16
                        nc.tensor.matmul(
                            ps,
                            xT_bf[:, kt, :],
                            win_bf[:, kt, :],
                            start=(kt == 0),
                            stop=(kt == D_MODEL // 128 - 1),
                        )
                    nc.scalar.activation(
                        out=proj_sb[:, ci * 512:(ci + 1) * 512],
                        in_=ps,
                        func=mybir.ActivationFunctionType.Copy,
                    )

            # views into proj
            q_content = proj_sb[:, 0: H * C]                    # [4,1024]
            q_rope_raw = proj_sb[:, H * C: H * C + H * R]       # [4,1024]
            ckv_new = proj_sb[:, 2048: 2048 + RANK]             # [4,512]
            k_rope_raw = proj_sb[:, 2560: 2560 + H * R]         # [4,1024]

            # ---------- stage 2: rope rotation for q_rope and k_rope_new ----------
            rot_q = singles.tile([T, H * R], F32)
            rot_k = singles.tile([T, H * R], F32)
            tmpA = small.tile([T, H * R], F32, tag="ropetmp")
            tmpB = small.tile([T, H * R], F32, tag="ropetmp")
            _rope(nc, rot_q, q_rope_raw, tmpA, tmpB, ctab_sb, stab_sb)
            tmpA2 = small.tile([T, H * R], F32, tag="ropetmp")
            tmpB2 = small.tile([T, H * R], F32, tag="ropetmp")
            _rope(nc, rot_k, k_rope_raw, tmpA2, tmpB2, ctab_sb, stab_sb)

            # bf16 casts of the new-token tensors (transposed via PE below)
            qc_bf = singles.tile([T, H * C], BF16)
            nc.vector.tensor_copy(out=qc_bf, in_=q_content)
            rotq_bf = singles.tile([T, H * R], BF16)
            nc.vector.tensor_copy(out=rotq_bf, in_=rot_q)
            rotk_bf = singles.tile([T, H * R], BF16)
            nc.vector.tensor_copy(out=rotk_bf, in_=rot_k)
            ckvnew_bf = singles.tile([T, RANK], BF16)
            nc.vector.tensor_copy(out=ckvnew_bf, in_=ckv_new)

            # ---------- stages 3-5: transposed new-token tensors -----------------
            # Q_augT [1536, 64]: col (h,t) = 4h+t; rows 0..511 = q_abs,
            # rows 512+64h.. = q_rope[h].  z_newT [1536, 4] likewise holds the
            # new tokens' latent (c_kv_new ; k_rope_new).
            qaugT_bf = singles.tile([128, ZKT, 64], BF16)
            nc.vector.memset(qaugT_bf, 0.0)
            znewT_bf = singles.tile([128, ZKT, T], BF16)

            # q_contentT per head at base partition 0: even heads come out of
            # the transpose on partitions 0..63 (DVE copy); odd heads land on
            # partitions 64..127 and are moved down with an SBUF->SBUF DMA.
            qcT = singles.tile([C, H, T], BF16)     # [64,16,4]
            qcT2 = singles.tile([128, H * C // 128, T], BF16)  # [128,8,4]
            for j in range(H * C // 128):  # 8 q_content transposes
                pt = tp_psum.tile([128, T], BF16, tag="tp")
                nc.tensor.transpose(
                    pt, qc_bf[:, j * 128:(j + 1) * 128], id_bf[0:T, 0:T]
                )
                nc.vector.tensor_copy(out=qcT2[:, j, :], in_=pt)
                nc.vector.tensor_copy(out=qcT[:, 2 * j, :], in_=pt[0:C, :])
            for j in range(H * C // 128):
                nc.sync.dma_start(
                    out=qcT[:, 2 * j + 1, :], in_=qcT2[C: 2 * C, j, :]
                )
            for j in range(RANK // 128):  # c_kv_new transposes -> znewT rows 0..511
                pt = tp_psum.tile([128, T], BF16, tag="tp")
                nc.tensor.transpose(
                    pt, ckvnew_bf[:, j * 128:(j + 1) * 128], id_bf[0:T, 0:T]
                )
                nc.vector.tensor_copy(out=znewT_bf[:, j, :], in_=pt)
            for j in range(H * R // 128):  # q_rope transposes -> qaugT rope rows
                pt = tp_psum.tile([128, T], BF16, tag="tp")
                nc.tensor.transpose(
                    pt, rotq_bf[:, j * 128:(j + 1) * 128], id_bf[0:T, 0:T]
                )
                nc.vector.tensor_copy(
                    out=qaugT_bf[0:64, 4 + j, 8 * j: 8 * j + 4], in_=pt[0:64, :]
                )
                nc.vector.tensor_copy(
                    out=qaugT_bf[64:128, 4 + j, 8 * j + 4: 8 * j + 8],
                    in_=pt[64:128, :],
                )
            for j in range(H * R // 128):  # k_rope_new transposes -> znewT rope
                pt = tp_psum.tile([128, T], BF16, tag="tp")
                nc.tensor.transpose(
                    pt, rotk_bf[:, j * 128:(j + 1) * 128], id_bf[0:T, 0:T]
                )
                nc.vector.tensor_copy(out=znewT_bf[:, 4 + j, :], in_=pt)

            # absorbed queries: q_abs[h] = q_content[h] @ w_up_tc[h] -> rows 0..511
            with tc.tile_pool(name="qabs_psum", bufs=2, space="PSUM") as qabs_psum:
                for rt in range(RANK // 128):  # 4 r-tiles
                    ps = qabs_psum.tile([128, 64], F32)
                    for h in range(H):
                        nc.tensor.matmul(
                            ps[:, 4 * h: 4 * h + 4],
                            wuptc_sb[
                                :, h * RANK + rt * 128: h * RANK + (rt + 1) * 128
                            ],
                            qcT[:, h, :],
                            start=True,
                            stop=True,
                        )
                    nc.vector.tensor_copy(out=qaugT_bf[:, rt, :], in_=ps)

            # ---------- stage 6: scores + exp + row-sums ------------------------
            exp_sb = singles.tile([64, S], BF16)
            sums = singles.tile([64, NCHUNKS + 1], F32)
            with tc.tile_pool(name="sc_psum", bufs=2, space="PSUM") as sc_psum:
                for ci in range(NCHUNKS):  # 8 x 512 cache positions
                    zt_bf = zt_pool.tile([128, ZKT, SCHUNK], BF16, tag="zt_bf")
                    nc.sync.dma_start(
                        out=zt_bf, in_=d_zt[ci * 128:(ci + 1) * 128]
                    )
                    ps = sc_psum.tile([64, SCHUNK], F32, tag="sc")
                    for kt in range(ZKT):
                        nc.tensor.matmul(
                            ps,
                            qaugT_bf[:, kt, :],
                            zt_bf[:, kt, :],
                            start=(kt == 0),
                            stop=(kt == ZKT - 1),
                        )
                    nc.scalar.activation(
                        out=exp_sb[:, ci * SCHUNK:(ci + 1) * SCHUNK],
                        in_=ps,
                        func=mybir.ActivationFunctionType.Exp,
                        accum_out=sums[:, ci: ci + 1],
                    )
                # new-token chunk (4 cols)
                ps9 = sc_psum.tile([64, T], F32, tag="sc9")
                for kt in range(ZKT):
                    nc.tensor.matmul(
                        ps9,
                        qaugT_bf[:, kt, :],
                        znewT_bf[:, kt, :],
                        start=(kt == 0),
                        stop=(kt == ZKT - 1),
                    )
                nc.scalar.activation(
                    out=exp_sb[:, S0:S],
                    in_=ps9,
                    func=mybir.ActivationFunctionType.Exp,
                    accum_out=sums[:, NCHUNKS: NCHUNKS + 1],
                )

            # ---------- stage 7: softmax normalize ------------------------------
            total = singles.tile([64, 1], F32)
            recip = singles.tile([64, 1], F32)
            nc.vector.reduce_sum(out=total, in_=sums, axis=mybir.AxisListType.X)
            nc.vector.reciprocal(out=recip, in_=total)
            nc.vector.tensor_scalar_mul(exp_sb, exp_sb, recip)

            # ---------- stage 8: transpose attn -> attnT [4100, 64] -------------
            attnT = singles.tile([128, S0 // 128 + 1, 64], BF16)  # [128,33,64]
            for st in range(S0 // 128):  # 32
                pt = tp_psum.tile([128, 64], BF16, tag="tp")
                nc.tensor.transpose(
                    pt, exp_sb[:, st * 128:(st + 1) * 128], id_bf[0:64, 0:64]
                )
                nc.vector.tensor_copy(out=attnT[:, st, :], in_=pt)
            ptn = tp_psum.tile([T, 64], BF16, tag="tpn")
            nc.tensor.transpose(ptn, exp_sb[:, S0:S], id_bf[0:64, 0:64])
            nc.vector.tensor_copy(out=attnT[0:T, S0 // 128, :], in_=ptn)

            # ---------- stage 9: o_lat = attn @ c_kv  [64, 512] ------------------
            o_lat = singles.tile([64, RANK], F32)
            with tc.tile_pool(name="olat_psum", bufs=1, space="PSUM") as olat_psum:
                pso = olat_psum.tile([64, RANK], F32)
                for kg in range(NCHUNKS):
                    ckv_bf = ckv_pool.tile([128, 4, RANK], BF16, tag="ckv_bf")
                    nc.sync.dma_start(
                        out=ckv_bf, in_=d_ckv[kg * 128:(kg + 1) * 128]
                    )
                    for kt in range(4):
                        nc.tensor.matmul(
                            pso,
                            attnT[:, kg * 4 + kt, :],
                            ckv_bf[:, kt, :],
                            start=(kg == 0 and kt == 0),
                            stop=False,
                        )
                nc.tensor.matmul(
                    pso,
                    attnT[0:T, S0 // 128, :],
                    ckvnew_bf,
                    start=False,
                    stop=True,
                )
                nc.vector.tensor_copy(out=o_lat, in_=pso)

            # o_latT [512, 64] in bf16
            olat_bf = singles.tile([64, RANK], BF16)
            nc.vector.tensor_copy(out=olat_bf, in_=o_lat)
            olatT = singles.tile([128, RANK // 128, 64], BF16)
            for rt in range(RANK // 128):
                pt = tp_psum.tile([128, 64], BF16, tag="tp")
                nc.tensor.transpose(
                    pt, olat_bf[:, rt * 128:(rt + 1) * 128], id_bf[0:64, 0:64]
                )
                nc.vector.tensor_copy(out=olatT[:, rt, :], in_=pt)

            # ---------- stage 10: attn_outT [2048, 4] = blockdiag(w_up^T)@o_latT
            aoT = singles.tile([128, H, T], BF16)
            with tc.tile_pool(name="ao_psum", bufs=2, space="PSUM") as ao_psum:
                for h in range(H):
                    ps = ao_psum.tile([128, T], F32)
                    for rt in range(RANK // 128):
                        nc.tensor.matmul(
                            ps,
                            wup_sb[:, rt, h * D:(h + 1) * D],
                            olatT[:, rt, 4 * h: 4 * h + 4],
                            start=(rt == 0),
                            stop=(rt == RANK // 128 - 1),
                        )
                    nc.vector.tensor_copy(out=aoT[:, h, :], in_=ps)

            # ---------- stage 11: out = attn_out @ w_out [4, 2048] ---------------
            out_stage = singles.tile([T, D_MODEL], F32)
            with tc.tile_pool(name="out_psum", bufs=1, space="PSUM") as out_psum:
                pouts = [
                    out_psum.tile([T, 512], F32, name=f"pout{i}", tag=f"pout{i}")
                    for i in range(4)
                ]
                for kt in range(H * D // 128):  # 16
                    wout_bf = wout_pool.tile([128, D_MODEL], BF16, tag="wout_bf")
                    nc.sync.dma_start(
                        out=wout_bf, in_=d_wout[kt * 128:(kt + 1) * 128, :]
                    )
                    for ci in range(4):
                        # D == 128, so attn_outT k-tile kt is exactly head kt
                        nc.tensor.matmul(
                            pouts[ci],
                            aoT[:, kt, :],
                            wout_bf[:, ci * 512:(ci + 1) * 512],
                            start=(kt == 0),
                            stop=(kt == H * D // 128 - 1),
                        )
                for ci in range(4):
                    nc.vector.tensor_copy(
                        out=out_stage[:, ci * 512:(ci + 1) * 512], in_=pouts[ci]
                    )
                nc.sync.dma_start(out=d_out, in_=out_stage)

    nc.compile()
    return nc


def _rope(nc, rot, raw, tmpA, tmpB, ctab_sb, stab_sb):
    """rot[t, h*64+j]      = raw1*cos - raw2*sin   (j in [0,32))
       rot[t, h*64+32+j]   = raw1*sin + raw2*cos
    using A = raw*ctab, B = raw*stab with half-replicated cos/sin tables."""
    nc.vector.tensor_mul(tmpA, raw, ctab_sb)
    nc.vector.tensor_mul(tmpB, raw, stab_sb)
    a3 = tmpA.rearrange("p (h r) -> p h r", h=H)
    b3 = tmpB.rearrange("p (h r) -> p h r", h=H)
    r3 = rot.rearrange("p (h r) -> p h r", h=H)
    nc.vector.tensor_sub(r3[:, :, 0:32], a3[:, :, 0:32], b3[:, :, 32:64])
    nc.vector.tensor_add(r3[:, :, 32:64], b3[:, :, 0:32], a3[:, :, 32:64])


def _prep_shared(w_q, w_q_rope, w_kv_down, w_kv_up, w_k_rope, w_out):
    scale = np.float32(1.0 / np.sqrt(D))
    w_in = np.concatenate(
        [w_q * scale, w_q_rope * scale, w_kv_down, w_k_rope], axis=1
    )
    # chunked: [7*128, 16, 512] with row p of chunk ci holding
    # w_in[kt*128+p, ci*512:(ci+1)*512] for kt = 0..15 (contiguous /partition)
    w_in_c = np.ascontiguousarray(
        w_in.reshape(16, 128, PROJ_N // 512, 512).transpose(2, 1, 0, 3)
    ).reshape((PROJ_N // 512) * 128, 16, 512).astype(NPBF)
    w_up_c = np.ascontiguousarray(
        w_kv_up.reshape(RANK // 128, 128, H * D).transpose(1, 0, 2)
    ).astype(NPBF)                                    # [128, 4, 2048]
    w_up_tc = np.ascontiguousarray(
        w_kv_up.reshape(RANK, H, D)[:, :, :C].transpose(2, 1, 0)
    ).reshape(C, H * RANK).astype(NPBF)
    w_out_c = np.ascontiguousarray(w_out).astype(NPBF)
    pos = S0 + np.arange(T)
    invf = 1.0 / (10000.0 ** (np.arange(R // 2) / (R // 2)))
    f = (pos[:, None] * invf[None, :]).astype(np.float32)
    cos32, sin32 = np.cos(f).astype(np.float32), np.sin(f).astype(np.float32)
    ctab = np.ascontiguousarray(
        np.tile(np.concatenate([cos32, cos32], 1)[:, None, :], (1, H, 1))
    ).reshape(T, H * R)
    stab = np.ascontiguousarray(
        np.tile(np.concatenate([sin32, sin32], 1)[:, None, :], (1, H, 1))
    ).reshape(T, H * R)
    return w_in_c, w_up_c, w_up_tc, w_out_c, ctab, stab


def _make_in_maps(inputs):
    x = np.asarray(inputs["x"], np.float32)
    cache_c_kv = np.asarray(inputs["cache_c_kv"], np.float32)
    cache_k_rope = np.asarray(inputs["cache_k_rope"], np.float32)
    w_in_c, w_up_c, w_up_tc, w_out_c, ctab, stab = _prep_shared(
        np.asarray(inputs["w_q"], np.float32),
        np.asarray(inputs["w_q_rope"], np.float32),
        np.asarray(inputs["w_kv_down"], np.float32),
        np.asarray(inputs["w_kv_up"], np.float32),
        np.asarray(inputs["w_k_rope"], np.float32),
        np.asarray(inputs["w_out"], np.float32))

    in_maps = []
    for b in range(B):
        zt = np.concatenate(
            [cache_c_kv[b].T,
             cache_k_rope[b].transpose(1, 2, 0).reshape(H * R, S0)],
            axis=0,
        )  # [1536, 4096]
        # chunked: [8*128, 12, 512], chunk ci row p = z_t[kt*128+p, ci*512:...]
        zt_c = np.ascontiguousarray(
            zt.reshape(ZKT, 128, NCHUNKS, SCHUNK).transpose(2, 1, 0, 3)
        ).reshape(NCHUNKS * 128, ZKT, SCHUNK).astype(NPBF)
        # chunked row-major c_kv: [8*128, 4, 512], s = kg*512 + kt*128 + p
        ckv_c = np.ascontiguousarray(
            cache_c_kv[b].reshape(NCHUNKS, 4, 128, RANK).transpose(0, 2, 1, 3)
        ).reshape(NCHUNKS * 128, 4, RANK).astype(NPBF)
        xt_c = np.ascontiguousarray(
            x[b].T.reshape(D_MODEL // 128, 128, T).transpose(1, 0, 2)
        ).astype(NPBF)  # [128, 16, 4]
        in_maps.append({
            "xT": xt_c,
            "w_in": w_in_c,
            "w_up": w_up_c,
            "w_up_tc": w_up_tc,
            "w_out": w_out_c,
            "z_t": zt_c,
            "c_kv": ckv_c,
            "ctab": ctab,
            "stab": stab,
        })
    return in_maps


def kernel(x, cache_c_kv, cache_k_rope, w_q, w_q_rope, w_kv_down, w_kv_up,
           w_k_rope, w_out):
    global _BUILT, _LAST_RESULTS
    if _BUILT is None:
        _BUILT = build_bass()
    nc = _BUILT

    in_maps = _make_in_maps(dict(
        x=x, cache_c_kv=cache_c_kv, cache_k_rope=cache_k_rope, w_q=w_q,
        w_q_rope=w_q_rope, w_kv_down=w_kv_down, w_kv_up=w_kv_up,
        w_k_rope=w_k_rope, w_out=w_out))

    res = bass_utils.run_bass_kernel_spmd(nc, in_maps, core_ids=list(range(B)))
    _LAST_RESULTS = res
    return np.stack([res.results[b]["out"] for b in range(B)]).astype(np.float32)


# revision 9
# speedup vs baseline: 2.2388x; 1.1360x over previous
"""DeepSeek MLA decode-step kernel for 8 Trainium2 NeuronCores.

Strategy
--------
- Data-parallel over batch: B=8 -> one batch element per core, no collectives.
- MLA weight absorption: the latent cache is never decompressed. Scores are
  computed against an augmented latent z = [c_kv(512) ; k_rope per head(16*64)]
  (1536 dims) with a block-sparse augmented query Q_aug built on-chip:
      scores[(h,t), s] = q_abs[h,t,:512] . c_kv[s] + q_rope[h,t] . k_rope[s,h]
  where q_abs = q_content @ w_kv_up[:, h, :C].T  (absorbed).
  The attention output stays in latent space (o_lat = attn @ c_kv), is pushed
  through w_kv_up per head, then through w_out.
- All large operands are cast to bf16 on the host and shipped pre-chunked so
  every DMA lands contiguous per partition on hardware DGE queues (no in-
  flight dtype-casting SWDGE, which caps at ~12 B/ns/engine).
- Per-core HBM traffic ~43 MB -> memory-bound target ~130-150 us.
"""

import numpy as np
import ml_dtypes

import concourse.bass as bass
import concourse.mybir as mybir
import concourse.tile as tile
from concourse import bacc
from concourse import bass_utils
from concourse.masks import make_identity

# Problem dims (hardcoded per contract)
B, T, S0 = 8, 4, 4096
D_MODEL, H, D, R, RANK = 2048, 16, 128, 64, 512
C = D - R                  # 64 content dims per head
S = S0 + T                 # 4100 total positions
Z = RANK + H * R           # 1536 augmented latent dim
ZKT = Z // 128             # 12 k-tiles over latent dim
PROJ_N = H * C + H * R + RANK + H * R   # 3584 fused projection cols
SCHUNK = 512               # score-chunk width over cache positions
NCHUNKS = S0 // SCHUNK     # 8
F32 = mybir.dt.float32
BF16 = mybir.dt.bfloat16
NPBF = ml_dtypes.bfloat16

_BUILT = None  # (nc,) cached across calls in one process
_LAST_RESULTS = None  # BassKernelResults of the most recent run (for test harness)


def build_bass():
    nc = bacc.Bacc("TRN2", target_bir_lowering=False, debug=False, num_devices=8)

    # ---- per-core DRAM I/O (bf16, host-prechunked contiguous layouts) ----
    d_xT = nc.dram_tensor("xT", [128, D_MODEL // 128, T], BF16,
                          kind="ExternalInput").ap()
    d_win = nc.dram_tensor("w_in", [(PROJ_N // 512) * 128, D_MODEL // 128, 512],
                           BF16, kind="ExternalInput").ap()
    d_wup = nc.dram_tensor("w_up", [128, RANK // 128, H * D], BF16,
                           kind="ExternalInput").ap()
    d_wuptc = nc.dram_tensor("w_up_tc", [C, H * RANK], BF16,
                             kind="ExternalInput").ap()
    d_wout = nc.dram_tensor("w_out", [H * D, D_MODEL], BF16,
                            kind="ExternalInput").ap()
    d_zt = nc.dram_tensor("z_t", [NCHUNKS * 128, ZKT, SCHUNK], BF16,
                          kind="ExternalInput").ap()
    d_ckv = nc.dram_tensor("c_kv", [NCHUNKS * 128, S0 // NCHUNKS // 128, RANK],
                           BF16, kind="ExternalInput").ap()
    d_ctab = nc.dram_tensor("ctab", [T, H * R], F32, kind="ExternalInput").ap()
    d_stab = nc.dram_tensor("stab", [T, H * R], F32, kind="ExternalInput").ap()
    d_out = nc.dram_tensor("out", [T, D_MODEL], F32, kind="ExternalOutput").ap()

    with tile.TileContext(nc) as tc:
        with (
            tc.tile_pool(name="singles", bufs=1) as singles,
            tc.tile_pool(name="small", bufs=2) as small,
            tc.tile_pool(name="win_pool", bufs=2) as win_pool,
            tc.tile_pool(name="zt_pool", bufs=4) as zt_pool,
            tc.tile_pool(name="ckv_pool", bufs=3) as ckv_pool,
            tc.tile_pool(name="wout_pool", bufs=3) as wout_pool,
            tc.tile_pool(name="tp_psum", bufs=2, space="PSUM") as tp_psum,
        ):
            # ---------- resident SBUF loads ----------
            identity = singles.tile([128, 128], F32)
            make_identity(nc, identity)
            id_bf = singles.tile([128, 128], BF16)
            nc.vector.tensor_copy(out=id_bf, in_=identity)

            # DMA issue order == consumption order: xT/tabs, wuptc (qabs,
            # ~45us), then win chunks (proj), zt/ckv chunks (scores/o_lat),
            # w_up (stage 10), w_out (stage 11). w_up/w_out issue late so the
            # first proj chunk lands ~12us after launch instead of ~30us.
            xT_bf = singles.tile([128, D_MODEL // 128, T], BF16)
            nc.sync.dma_start(out=xT_bf, in_=d_xT)
            ctab_sb = singles.tile([T, H * R], F32)
            nc.sync.dma_start(out=ctab_sb, in_=d_ctab)
            stab_sb = singles.tile([T, H * R], F32)
            nc.sync.dma_start(out=stab_sb, in_=d_stab)
            wuptc_sb = singles.tile([C, H * RANK], BF16)  # [64, 8192]
            nc.sync.dma_start(out=wuptc_sb, in_=d_wuptc)
            wup_sb = singles.tile([128, RANK // 128, H * D], BF16)

            # ---------- stage 1: fused input projection  proj = x @ W_in ----------
            proj_sb = singles.tile([T, PROJ_N], F32)
            with tc.tile_pool(name="proj_psum", bufs=2, space="PSUM") as proj_psum:
                for ci in range(PROJ_N // 512):  # 7 chunks
                    win_bf = win_pool.tile([128, D_MODEL // 128, 512], BF16,
                                           tag="win_bf")
                    nc.sync.dma_start(
                        out=win_bf, in_=d_win[ci * 128:(ci + 1) * 128]
                    )
                    ps = proj_psum.tile([T, 512], F32)
                    for kt in range(D_MODEL // 128):  # 16
                        nc.tensor.matmul(
                            ps,
                            xT_bf[:, kt, :],
                            win_bf[:, kt, :],
                            start=(kt == 0),
                            stop=(kt == D_MODEL // 128 - 1),
                        )
                    nc.scalar.activation(
                        out=proj_sb[:, ci * 512:(ci + 1) * 512],
                        in_=ps,
                        func=mybir.ActivationFunctionType.Copy,
                    )

            # views into proj
            q_content = proj_sb[:, 0: H * C]                    # [4,1024]
            q_rope_raw = proj_sb[:, H * C: H * C + H * R]       # [4,1024]
            ckv_new = proj_sb[:, 2048: 2048 + RANK]             # [4,512]
            k_rope_raw = proj_sb[:, 2560: 2560 + H * R]         # [4,1024]

            # ---------- stage 2: rope rotation for q_rope and k_rope_new ----------
            rot_q = singles.tile([T, H * R], F32)
            rot_k = singles.tile([T, H * R], F32)
            tmpA = small.tile([T, H * R], F32, tag="ropetmp")
            tmpB = small.tile([T, H * R], F32, tag="ropetmp")
            _rope(nc, rot_q, q_rope_raw, tmpA, tmpB, ctab_sb, stab_sb)
            tmpA2 = small.tile([T, H * R], F32, tag="ropetmp")
            tmpB2 = small.tile([T, H * R], F32, tag="ropetmp")
            _rope(nc, rot_k, k_rope_raw, tmpA2, tmpB2, ctab_sb, stab_sb)

            # bf16 casts of the new-token tensors (transposed via PE below)
            qc_bf = singles.tile([T, H * C], BF16)
            nc.vector.tensor_copy(out=qc_bf, in_=q_content)
            rotq_bf = singles.tile([T, H * R], BF16)
            nc.vector.tensor_copy(out=rotq_bf, in_=rot_q)
            rotk_bf = singles.tile([T, H * R], BF16)
            nc.vector.tensor_copy(out=rotk_bf, in_=rot_k)
            ckvnew_bf = singles.tile([T, RANK], BF16)
            nc.vector.tensor_copy(out=ckvnew_bf, in_=ckv_new)

            # ---------- stages 3-5: transposed new-token tensors -----------------
            # Q_augT [1536, 64]: col (h,t) = 4h+t; rows 0..511 = q_abs,
            # rows 512+64h.. = q_rope[h].  z_newT [1536, 4] likewise holds the
            # new tokens' latent (c_kv_new ; k_rope_new).
            qaugT_bf = singles.tile([128, ZKT, 64], BF16)
            nc.vector.memset(qaugT_bf, 0.0)
            znewT_bf = singles.tile([128, ZKT, T], BF16)

            # q_contentT per head at base partition 0: even heads come out of
            # the transpose on partitions 0..63 (DVE copy); odd heads land on
            # partitions 64..127 and are moved down with an SBUF->SBUF DMA.
            qcT = singles.tile([C, H, T], BF16)     # [64,16,4]
            qcT2 = singles.tile([128, H * C // 128, T], BF16)  # [128,8,4]
            for j in range(H * C // 128):  # 8 q_content transposes
                pt = tp_psum.tile([128, T], BF16, tag="tp")
                nc.tensor.transpose(
                    pt, qc_bf[:, j * 128:(j + 1) * 128], id_bf[0:T, 0:T]
                )
                nc.vector.tensor_copy(out=qcT2[:, j, :], in_=pt)
                nc.vector.tensor_copy(out=qcT[:, 2 * j, :], in_=pt[0:C, :])
            for j in range(H * C // 128):
                nc.sync.dma_start(
                    out=qcT[:, 2 * j + 1, :], in_=qcT2[C: 2 * C, j, :]
                )
            for j in range(RANK // 128):  # c_kv_new transposes -> znewT rows 0..511
                pt = tp_psum.tile([128, T], BF16, tag="tp")
                nc.tensor.transpose(
                    pt, ckvnew_bf[:, j * 128:(j + 1) * 128], id_bf[0:T, 0:T]
                )
                nc.vector.tensor_copy(out=znewT_bf[:, j, :], in_=pt)
            for j in range(H * R // 128):  # q_rope transposes -> qaugT rope rows
                pt = tp_psum.tile([128, T], BF16, tag="tp")
                nc.tensor.transpose(
                    pt, rotq_bf[:, j * 128:(j + 1) * 128], id_bf[0:T, 0:T]
                )
                nc.vector.tensor_copy(
                    out=qaugT_bf[0:64, 4 + j, 8 * j: 8 * j + 4], in_=pt[0:64, :]
                )
                nc.vector.tensor_copy(
                    out=qaugT_bf[64:128, 4 + j, 8 * j + 4: 8 * j + 8],
                    in_=pt[64:128, :],
                )
            for j in range(H * R // 128):  # k_rope_new transposes -> znewT rope
                pt = tp_psum.tile([128, T], BF16, tag="tp")
                nc.tensor.transpose(
                    pt, rotk_bf[:, j * 128:(j + 1) * 128], id_bf[0:T, 0:T]
                )
                nc.vector.tensor_copy(out=znewT_bf[:, 4 + j, :], in_=pt)

            # absorbed queries: q_abs[h] = q_content[h] @ w_up_tc[h] -> rows 0..511
            with tc.tile_pool(name="qabs_psum", bufs=2, space="PSUM") as qabs_psum:
                for rt in range(RANK // 128):  # 4 r-tiles
                    ps = qabs_psum.tile([128, 64], F32)
                    for h in range(H):
                        nc.tensor.matmul(
                            ps[:, 4 * h: 4 * h + 4],
                            wuptc_sb[
                                :, h * RANK + rt * 128: h * RANK + (rt + 1) * 128
                            ],
                            qcT[:, h, :],
                            start=True,
                            stop=True,
                        )
                    nc.vector.tensor_copy(out=qaugT_bf[:, rt, :], in_=ps)

            # ---------- stages 6+8+9 fused: per 512-chunk of cache positions:
            # scores -> exp (unnormalized, accum row-sums) -> PE transpose ->
            # o_lat accumulation. Softmax normalization commutes with
            # attn @ c_kv, so it is applied to the tiny o_lat at the end.
            exp_sb = singles.tile([64, S], BF16)
            sums = singles.tile([64, NCHUNKS + 1], F32)
            attnT = singles.tile([128, S0 // 128 + 1, 64], BF16)  # [128,33,64]
            o_lat = singles.tile([64, RANK], F32)
            with (
                tc.tile_pool(name="sc_psum", bufs=2, space="PSUM") as sc_psum,
                tc.tile_pool(name="olat_psum", bufs=1, space="PSUM") as olat_psum,
            ):
                pso = olat_psum.tile([64, RANK], F32)
                for ci in range(NCHUNKS):  # 8 x 512 cache positions
                    zt_bf = zt_pool.tile([128, ZKT, SCHUNK], BF16, tag="zt_bf")
                    nc.sync.dma_start(
                        out=zt_bf, in_=d_zt[ci * 128:(ci + 1) * 128]
                    )
                    ckv_bf = ckv_pool.tile([128, 4, RANK], BF16, tag="ckv_bf")
                    nc.sync.dma_start(
                        out=ckv_bf, in_=d_ckv[ci * 128:(ci + 1) * 128]
                    )
                    ps = sc_psum.tile([64, SCHUNK], F32, tag="sc")
                    for kt in range(ZKT):
                        nc.tensor.matmul(
                            ps,
                            qaugT_bf[:, kt, :],
                            zt_bf[:, kt, :],
                            start=(kt == 0),
                            stop=(kt == ZKT - 1),
                        )
                    nc.scalar.activation(
                        out=exp_sb[:, ci * SCHUNK:(ci + 1) * SCHUNK],
                        in_=ps,
                        func=mybir.ActivationFunctionType.Exp,
                        accum_out=sums[:, ci: ci + 1],
                    )
                    for kt in range(4):
                        pt = tp_psum.tile([128, 64], BF16, tag="tp")
                        nc.tensor.transpose(
                            pt,
                            exp_sb[:, ci * 512 + kt * 128: ci * 512 + (kt + 1) * 128],
                            id_bf[0:64, 0:64],
                        )
                        nc.vector.tensor_copy(out=attnT[:, ci * 4 + kt, :], in_=pt)
                        nc.tensor.matmul(
                            pso,
                            attnT[:, ci * 4 + kt, :],
                            ckv_bf[:, kt, :],
                            start=(ci == 0 and kt == 0),
                            stop=False,
                        )
                # new-token chunk (4 cols)
                ps9 = sc_psum.tile([64, T], F32, tag="sc")
                for kt in range(ZKT):
                    nc.tensor.matmul(
                        ps9,
                        qaugT_bf[:, kt, :],
                        znewT_bf[:, kt, :],
                        start=(kt == 0),
                        stop=(kt == ZKT - 1),
                    )
                nc.scalar.activation(
                    out=exp_sb[:, S0:S],
                    in_=ps9,
                    func=mybir.ActivationFunctionType.Exp,
                    accum_out=sums[:, NCHUNKS: NCHUNKS + 1],
                )
                ptn = tp_psum.tile([T, 64], BF16, tag="tp")
                nc.tensor.transpose(ptn, exp_sb[:, S0:S], id_bf[0:64, 0:64])
                nc.vector.tensor_copy(out=attnT[0:T, S0 // 128, :], in_=ptn)
                nc.tensor.matmul(
                    pso,
                    attnT[0:T, S0 // 128, :],
                    ckvnew_bf,
                    start=False,
                    stop=True,
                )
                # softmax denominator, applied in latent space
                total = singles.tile([64, 1], F32)
                recip = singles.tile([64, 1], F32)
                nc.vector.reduce_sum(
                    out=total, in_=sums, axis=mybir.AxisListType.X
                )
                nc.vector.reciprocal(out=recip, in_=total)
                nc.vector.tensor_copy(out=o_lat, in_=pso)
                nc.vector.tensor_scalar_mul(o_lat, o_lat, recip)

            # w_up arrives behind the zt/ckv stream, well before stage 10
            nc.sync.dma_start(out=wup_sb, in_=d_wup)

            # o_latT [512, 64] in bf16
            olat_bf = singles.tile([64, RANK], BF16)
            nc.vector.tensor_copy(out=olat_bf, in_=o_lat)
            olatT = singles.tile([128, RANK // 128, 64], BF16)
            for rt in range(RANK // 128):
                pt = tp_psum.tile([128, 64], BF16, tag="tp")
                nc.tensor.transpose(
                    pt, olat_bf[:, rt * 128:(rt + 1) * 128], id_bf[0:64, 0:64]
                )
                nc.vector.tensor_copy(out=olatT[:, rt, :], in_=pt)

            # ---------- stage 10: attn_outT [2048, 4] = blockdiag(w_up^T)@o_latT
            aoT = singles.tile([128, H, T], BF16)
            with tc.tile_pool(name="ao_psum", bufs=2, space="PSUM") as ao_psum:
                for h in range(H):
                    ps = ao_psum.tile([128, T], F32)
                    for rt in range(RANK // 128):
                        nc.tensor.matmul(
                            ps,
                            wup_sb[:, rt, h * D:(h + 1) * D],
                            olatT[:, rt, 4 * h: 4 * h + 4],
                            start=(rt == 0),
                            stop=(rt == RANK // 128 - 1),
                        )
                    nc.vector.tensor_copy(out=aoT[:, h, :], in_=ps)

            # ---------- stage 11: out = attn_out @ w_out [4, 2048] ---------------
            out_stage = singles.tile([T, D_MODEL], F32)
            with tc.tile_pool(name="out_psum", bufs=1, space="PSUM") as out_psum:
                pouts = [
                    out_psum.tile([T, 512], F32, name=f"pout{i}", tag=f"pout{i}")
                    for i in range(4)
                ]
                for kt in range(H * D // 128):  # 16
                    wout_bf = wout_pool.tile([128, D_MODEL], BF16, tag="wout_bf")
                    nc.sync.dma_start(
                        out=wout_bf, in_=d_wout[kt * 128:(kt + 1) * 128, :]
                    )
                    for ci in range(4):
                        # D == 128, so attn_outT k-tile kt is exactly head kt
                        nc.tensor.matmul(
                            pouts[ci],
                            aoT[:, kt, :],
                            wout_bf[:, ci * 512:(ci + 1) * 512],
                            start=(kt == 0),
                            stop=(kt == H * D // 128 - 1),
                        )
                for ci in range(4):
                    nc.vector.tensor_copy(
                        out=out_stage[:, ci * 512:(ci + 1) * 512], in_=pouts[ci]
                    )
                nc.sync.dma_start(out=d_out, in_=out_stage)

    nc.compile()
    return nc


def _rope(nc, rot, raw, tmpA, tmpB, ctab_sb, stab_sb):
    """rot[t, h*64+j]      = raw1*cos - raw2*sin   (j in [0,32))
       rot[t, h*64+32+j]   = raw1*sin + raw2*cos
    using A = raw*ctab, B = raw*stab with half-replicated cos/sin tables."""
    nc.vector.tensor_mul(tmpA, raw, ctab_sb)
    nc.vector.tensor_mul(tmpB, raw, stab_sb)
    a3 = tmpA.rearrange("p (h r) -> p h r", h=H)
    b3 = tmpB.rearrange("p (h r) -> p h r", h=H)
    r3 = rot.rearrange("p (h r) -> p h r", h=H)
    nc.vector.tensor_sub(r3[:, :, 0:32], a3[:, :, 0:32], b3[:, :, 32:64])
    nc.vector.tensor_add(r3[:, :, 32:64], b3[:, :, 0:32], a3[:, :, 32:64])


def _prep_shared(w_q, w_q_rope, w_kv_down, w_kv_up, w_k_rope, w_out):
    scale = np.float32(1.0 / np.sqrt(D))
    w_in = np.concatenate(
        [w_q * scale, w_q_rope * scale, w_kv_down, w_k_rope], axis=1
    )
    # chunked: [7*128, 16, 512] with row p of chunk ci holding
    # w_in[kt*128+p, ci*512:(ci+1)*512] for kt = 0..15 (contiguous /partition)
    w_in_c = np.ascontiguousarray(
        w_in.reshape(16, 128, PROJ_N // 512, 512).transpose(2, 1, 0, 3)
    ).reshape((PROJ_N // 512) * 128, 16, 512).astype(NPBF)
    w_up_c = np.ascontiguousarray(
        w_kv_up.reshape(RANK // 128, 128, H * D).transpose(1, 0, 2)
    ).astype(NPBF)                                    # [128, 4, 2048]
    w_up_tc = np.ascontiguousarray(
        w_kv_up.reshape(RANK, H, D)[:, :, :C].transpose(2, 1, 0)
    ).reshape(C, H * RANK).astype(NPBF)
    w_out_c = np.ascontiguousarray(w_out).astype(NPBF)
    pos = S0 + np.arange(T)
    invf = 1.0 / (10000.0 ** (np.arange(R // 2) / (R // 2)))
    f = (pos[:, None] * invf[None, :]).astype(np.float32)
    cos32, sin32 = np.cos(f).astype(np.float32), np.sin(f).astype(np.float32)
    ctab = np.ascontiguousarray(
        np.tile(np.concatenate([cos32, cos32], 1)[:, None, :], (1, H, 1))
    ).reshape(T, H * R)
    stab = np.ascontiguousarray(
        np.tile(np.concatenate([sin32, sin32], 1)[:, None, :], (1, H, 1))
    ).reshape(T, H * R)
    return w_in_c, w_up_c, w_up_tc, w_out_c, ctab, stab


def _make_in_maps(inputs):
    x = np.asarray(inputs["x"], np.float32)
    cache_c_kv = np.asarray(inputs["cache_c_kv"], np.float32)
    cache_k_rope = np.asarray(inputs["cache_k_rope"], np.float32)
    w_in_c, w_up_c, w_up_tc, w_out_c, ctab, stab = _prep_shared(
        np.asarray(inputs["w_q"], np.float32),
        np.asarray(inputs["w_q_rope"], np.float32),
        np.asarray(inputs["w_kv_down"], np.float32),
        np.asarray(inputs["w_kv_up"], np.float32),
        np.asarray(inputs["w_k_rope"], np.float32),
        np.asarray(inputs["w_out"], np.float32))

    in_maps = []
    for b in range(B):
        zt = np.concatenate(
            [cache_c_kv[b].T,
             cache_k_rope[b].transpose(1, 2, 0).reshape(H * R, S0)],
            axis=0,
        )  # [1536, 4096]
        # chunked: [8*128, 12, 512], chunk ci row p = z_t[kt*128+p, ci*512:...]
        zt_c = np.ascontiguousarray(
            zt.reshape(ZKT, 128, NCHUNKS, SCHUNK).transpose(2, 1, 0, 3)
        ).reshape(NCHUNKS * 128, ZKT, SCHUNK).astype(NPBF)
        # chunked row-major c_kv: [8*128, 4, 512], s = kg*512 + kt*128 + p
        ckv_c = np.ascontiguousarray(
            cache_c_kv[b].reshape(NCHUNKS, 4, 128, RANK).transpose(0, 2, 1, 3)
        ).reshape(NCHUNKS * 128, 4, RANK).astype(NPBF)
        xt_c = np.ascontiguousarray(
            x[b].T.reshape(D_MODEL // 128, 128, T).transpose(1, 0, 2)
        ).astype(NPBF)  # [128, 16, 4]
        in_maps.append({
            "xT": xt_c,
            "w_in": w_in_c,
            "w_up": w_up_c,
            "w_up_tc": w_up_tc,
            "w_out": w_out_c,
            "z_t": zt_c,
            "c_kv": ckv_c,
            "ctab": ctab,
            "stab": stab,
        })
    return in_maps


def kernel(x, cache_c_kv, cache_k_rope, w_q, w_q_rope, w_kv_down, w_kv_up,
           w_k_rope, w_out):
    global _BUILT, _LAST_RESULTS
    if _BUILT is None:
        _BUILT = build_bass()
    nc = _BUILT

    in_maps = _make_in_maps(dict(
        x=x, cache_c_kv=cache_c_kv, cache_k_rope=cache_k_rope, w_q=w_q,
        w_q_rope=w_q_rope, w_kv_down=w_kv_down, w_kv_up=w_kv_up,
        w_k_rope=w_k_rope, w_out=w_out))

    res = bass_utils.run_bass_kernel_spmd(nc, in_maps, core_ids=list(range(B)))
    _LAST_RESULTS = res
    return np.stack([res.results[b]["out"] for b in range(B)]).astype(np.float32)


# revision 13
# speedup vs baseline: 2.2407x; 1.0008x over previous
"""DeepSeek MLA decode-step kernel for 8 Trainium2 NeuronCores.

Strategy
--------
- Data-parallel over batch: B=8 -> one batch element per core, no collectives.
- MLA weight absorption: the latent cache is never decompressed. Scores are
  computed against an augmented latent z = [c_kv(512) ; k_rope per head(16*64)]
  (1536 dims) with a block-sparse augmented query Q_aug built on-chip:
      scores[(h,t), s] = q_abs[h,t,:512] . c_kv[s] + q_rope[h,t] . k_rope[s,h]
  where q_abs = q_content @ w_kv_up[:, h, :C].T  (absorbed).
  The attention output stays in latent space (o_lat = attn @ c_kv), is pushed
  through w_kv_up per head, then through w_out.
- All large operands are cast to bf16 on the host and shipped pre-chunked so
  every DMA lands contiguous per partition on hardware DGE queues (no in-
  flight dtype-casting SWDGE, which caps at ~12 B/ns/engine).
- Per-core HBM traffic ~43 MB -> memory-bound target ~130-150 us.
"""

import numpy as np
import ml_dtypes

import concourse.bass as bass
import concourse.mybir as mybir
import concourse.tile as tile
from concourse import bacc
from concourse import bass_utils
from concourse.masks import make_identity

# Problem dims (hardcoded per contract)
B, T, S0 = 8, 4, 4096
D_MODEL, H, D, R, RANK = 2048, 16, 128, 64, 512
C = D - R                  # 64 content dims per head
S = S0 + T                 # 4100 total positions
Z = RANK + H * R           # 1536 augmented latent dim
ZKT = Z // 128             # 12 k-tiles over latent dim
PROJ_N = H * C + H * R + RANK + H * R   # 3584 fused projection cols
SCHUNK = 512               # score-chunk width over cache positions
NCHUNKS = S0 // SCHUNK     # 8
F32 = mybir.dt.float32
BF16 = mybir.dt.bfloat16
NPBF = ml_dtypes.bfloat16

_BUILT = None  # (nc,) cached across calls in one process
_LAST_RESULTS = None  # BassKernelResults of the most recent run (for test harness)


def build_bass():
    nc = bacc.Bacc("TRN2", target_bir_lowering=False, debug=False, num_devices=8)

    # ---- per-core DRAM I/O (bf16, host-prechunked contiguous layouts) ----
    d_xT = nc.dram_tensor("xT", [128, D_MODEL // 128, T], BF16,
                          kind="ExternalInput").ap()
    d_win = nc.dram_tensor("w_in", [(PROJ_N // 512) * 128, D_MODEL // 128, 512],
                           BF16, kind="ExternalInput").ap()
    d_wup = nc.dram_tensor("w_up", [128, RANK // 128, H * D], BF16,
                           kind="ExternalInput").ap()
    d_wuptc = nc.dram_tensor("w_up_tc", [C, H * RANK], BF16,
                             kind="ExternalInput").ap()
    d_wout = nc.dram_tensor("w_out", [H * D, D_MODEL], BF16,
                            kind="ExternalInput").ap()
    d_zt = nc.dram_tensor("z_t", [NCHUNKS * 128, ZKT, SCHUNK], BF16,
                          kind="ExternalInput").ap()
    d_ckv = nc.dram_tensor("c_kv", [NCHUNKS * 128, S0 // NCHUNKS // 128, RANK],
                           BF16, kind="ExternalInput").ap()
    d_ctab = nc.dram_tensor("ctab", [T, H * R], F32, kind="ExternalInput").ap()
    d_stab = nc.dram_tensor("stab", [T, H * R], F32, kind="ExternalInput").ap()
    d_out = nc.dram_tensor("out", [T, D_MODEL], F32, kind="ExternalOutput").ap()

    with tile.TileContext(nc) as tc:
        with (
            tc.tile_pool(name="singles", bufs=1) as singles,
            tc.tile_pool(name="small", bufs=2) as small,
            tc.tile_pool(name="tp_psum", bufs=2, space="PSUM") as tp_psum,
        ):
            # ---------- resident SBUF loads ----------
            identity = singles.tile([128, 128], F32)
            make_identity(nc, identity)
            id_bf = singles.tile([128, 128], BF16)
            nc.vector.tensor_copy(out=id_bf, in_=identity)

            # DMA issue order == consumption order: xT/tabs, wuptc (qabs,
            # ~45us), then win chunks (proj), zt/ckv chunks (scores/o_lat),
            # w_up (stage 10), w_out (stage 11). w_up/w_out issue late so the
            # first proj chunk lands ~12us after launch instead of ~30us.
            xT_bf = singles.tile([128, D_MODEL // 128, T], BF16)
            nc.sync.dma_start(out=xT_bf, in_=d_xT)
            ctab_sb = singles.tile([T, H * R], F32)
            nc.sync.dma_start(out=ctab_sb, in_=d_ctab)
            stab_sb = singles.tile([T, H * R], F32)
            nc.sync.dma_start(out=stab_sb, in_=d_stab)
            wuptc_sb = singles.tile([C, H * RANK], BF16)  # [64, 8192]
            nc.sync.dma_start(out=wuptc_sb, in_=d_wuptc)
            wup_sb = singles.tile([128, RANK // 128, H * D], BF16)

            # ---------- stage 1: fused input projection  proj = x @ W_in ----------
            proj_sb = singles.tile([T, PROJ_N], F32)
            with (
                tc.tile_pool(name="win_pool", bufs=2) as win_pool,
                tc.tile_pool(name="proj_psum", bufs=2, space="PSUM") as proj_psum,
            ):
                for ci in range(PROJ_N // 512):  # 7 chunks
                    win_bf = win_pool.tile([128, D_MODEL // 128, 512], BF16,
                                           tag="win_bf")
                    nc.sync.dma_start(
                        out=win_bf, in_=d_win[ci * 128:(ci + 1) * 128]
                    )
                    ps = proj_psum.tile([T, 512], F32)
                    for kt in range(D_MODEL // 128):  # 16
                        nc.tensor.matmul(
                            ps,
                            xT_bf[:, kt, :],
                            win_bf[:, kt, :],
                            start=(kt == 0),
                            stop=(kt == D_MODEL // 128 - 1),
                        )
                    nc.scalar.activation(
                        out=proj_sb[:, ci * 512:(ci + 1) * 512],
                        in_=ps,
                        func=mybir.ActivationFunctionType.Copy,
                    )

            # views into proj
            q_content = proj_sb[:, 0: H * C]                    # [4,1024]
            q_rope_raw = proj_sb[:, H * C: H * C + H * R]       # [4,1024]
            ckv_new = proj_sb[:, 2048: 2048 + RANK]             # [4,512]
            k_rope_raw = proj_sb[:, 2560: 2560 + H * R]         # [4,1024]

            # ---------- stage 2: rope rotation for q_rope and k_rope_new ----------
            rot_q = singles.tile([T, H * R], F32)
            rot_k = singles.tile([T, H * R], F32)
            tmpA = small.tile([T, H * R], F32, tag="ropetmp")
            tmpB = small.tile([T, H * R], F32, tag="ropetmp")
            _rope(nc, rot_q, q_rope_raw, tmpA, tmpB, ctab_sb, stab_sb)
            tmpA2 = small.tile([T, H * R], F32, tag="ropetmp")
            tmpB2 = small.tile([T, H * R], F32, tag="ropetmp")
            _rope(nc, rot_k, k_rope_raw, tmpA2, tmpB2, ctab_sb, stab_sb)

            # bf16 casts of the new-token tensors (transposed via PE below)
            qc_bf = singles.tile([T, H * C], BF16)
            nc.vector.tensor_copy(out=qc_bf, in_=q_content)
            rotq_bf = singles.tile([T, H * R], BF16)
            nc.vector.tensor_copy(out=rotq_bf, in_=rot_q)
            rotk_bf = singles.tile([T, H * R], BF16)
            nc.vector.tensor_copy(out=rotk_bf, in_=rot_k)
            ckvnew_bf = singles.tile([T, RANK], BF16)
            nc.vector.tensor_copy(out=ckvnew_bf, in_=ckv_new)

            # ---------- stages 3-5: transposed new-token tensors -----------------
            # Q_augT [1536, 64]: col (h,t) = 4h+t; rows 0..511 = q_abs,
            # rows 512+64h.. = q_rope[h].  z_newT [1536, 4] likewise holds the
            # new tokens' latent (c_kv_new ; k_rope_new).
            qaugT_bf = singles.tile([128, ZKT, 64], BF16)
            nc.vector.memset(qaugT_bf, 0.0)
            znewT_bf = singles.tile([128, ZKT, T], BF16)

            # q_contentT per head at base partition 0: even heads come out of
            # the transpose on partitions 0..63 (DVE copy); odd heads land on
            # partitions 64..127 and are moved down with an SBUF->SBUF DMA.
            qcT = singles.tile([C, H, T], BF16)     # [64,16,4]
            qcT2 = singles.tile([128, H * C // 128, T], BF16)  # [128,8,4]
            for j in range(H * C // 128):  # 8 q_content transposes
                pt = tp_psum.tile([128, T], BF16, tag="tp")
                nc.tensor.transpose(
                    pt, qc_bf[:, j * 128:(j + 1) * 128], id_bf[0:T, 0:T]
                )
                nc.vector.tensor_copy(out=qcT2[:, j, :], in_=pt)
                nc.vector.tensor_copy(out=qcT[:, 2 * j, :], in_=pt[0:C, :])
            for j in range(H * C // 128):
                nc.sync.dma_start(
                    out=qcT[:, 2 * j + 1, :], in_=qcT2[C: 2 * C, j, :]
                )
            for j in range(RANK // 128):  # c_kv_new transposes -> znewT rows 0..511
                pt = tp_psum.tile([128, T], BF16, tag="tp")
                nc.tensor.transpose(
                    pt, ckvnew_bf[:, j * 128:(j + 1) * 128], id_bf[0:T, 0:T]
                )
                nc.vector.tensor_copy(out=znewT_bf[:, j, :], in_=pt)
            for j in range(H * R // 128):  # q_rope transposes -> qaugT rope rows
                pt = tp_psum.tile([128, T], BF16, tag="tp")
                nc.tensor.transpose(
                    pt, rotq_bf[:, j * 128:(j + 1) * 128], id_bf[0:T, 0:T]
                )
                nc.vector.tensor_copy(
                    out=qaugT_bf[0:64, 4 + j, 8 * j: 8 * j + 4], in_=pt[0:64, :]
                )
                nc.vector.tensor_copy(
                    out=qaugT_bf[64:128, 4 + j, 8 * j + 4: 8 * j + 8],
                    in_=pt[64:128, :],
                )
            for j in range(H * R // 128):  # k_rope_new transposes -> znewT rope
                pt = tp_psum.tile([128, T], BF16, tag="tp")
                nc.tensor.transpose(
                    pt, rotk_bf[:, j * 128:(j + 1) * 128], id_bf[0:T, 0:T]
                )
                nc.vector.tensor_copy(out=znewT_bf[:, 4 + j, :], in_=pt)

            # absorbed queries: q_abs[h] = q_content[h] @ w_up_tc[h] -> rows 0..511
            with tc.tile_pool(name="qabs_psum", bufs=2, space="PSUM") as qabs_psum:
                for rt in range(RANK // 128):  # 4 r-tiles
                    ps = qabs_psum.tile([128, 64], F32)
                    for h in range(H):
                        nc.tensor.matmul(
                            ps[:, 4 * h: 4 * h + 4],
                            wuptc_sb[
                                :, h * RANK + rt * 128: h * RANK + (rt + 1) * 128
                            ],
                            qcT[:, h, :],
                            start=True,
                            stop=True,
                        )
                    nc.vector.tensor_copy(out=qaugT_bf[:, rt, :], in_=ps)

            # ---------- stages 6+8+9 fused: per 512-chunk of cache positions:
            # scores -> exp (unnormalized, accum row-sums) -> PE transpose ->
            # o_lat accumulation. Softmax normalization commutes with
            # attn @ c_kv, so it is applied to the tiny o_lat at the end.
            exp_sb = singles.tile([64, S], BF16)
            sums = singles.tile([64, NCHUNKS + 1], F32)
            attnT = singles.tile([128, S0 // 128 + 1, 64], BF16)  # [128,33,64]
            o_lat = singles.tile([64, RANK], F32)
            with (
                tc.tile_pool(name="zt_pool", bufs=4) as zt_pool,
                tc.tile_pool(name="ckv_pool", bufs=3) as ckv_pool,
                tc.tile_pool(name="sc_psum", bufs=2, space="PSUM") as sc_psum,
                tc.tile_pool(name="olat_psum", bufs=1, space="PSUM") as olat_psum,
            ):
                pso = olat_psum.tile([64, RANK], F32)
                for ci in range(NCHUNKS):  # 8 x 512 cache positions
                    zt_bf = zt_pool.tile([128, ZKT, SCHUNK], BF16, tag="zt_bf")
                    nc.sync.dma_start(
                        out=zt_bf, in_=d_zt[ci * 128:(ci + 1) * 128]
                    )
                    ckv_bf = ckv_pool.tile([128, 4, RANK], BF16, tag="ckv_bf")
                    nc.sync.dma_start(
                        out=ckv_bf, in_=d_ckv[ci * 128:(ci + 1) * 128]
                    )
                    ps = sc_psum.tile([64, SCHUNK], F32, tag="sc")
                    for kt in range(ZKT):
                        nc.tensor.matmul(
                            ps,
                            qaugT_bf[:, kt, :],
                            zt_bf[:, kt, :],
                            start=(kt == 0),
                            stop=(kt == ZKT - 1),
                        )
                    nc.scalar.activation(
                        out=exp_sb[:, ci * SCHUNK:(ci + 1) * SCHUNK],
                        in_=ps,
                        func=mybir.ActivationFunctionType.Exp,
                        accum_out=sums[:, ci: ci + 1],
                    )
                    for kt in range(4):
                        pt = tp_psum.tile([128, 64], BF16, tag="tp")
                        nc.tensor.transpose(
                            pt,
                            exp_sb[:, ci * 512 + kt * 128: ci * 512 + (kt + 1) * 128],
                            id_bf[0:64, 0:64],
                        )
                        nc.vector.tensor_copy(out=attnT[:, ci * 4 + kt, :], in_=pt)
                        nc.tensor.matmul(
                            pso,
                            attnT[:, ci * 4 + kt, :],
                            ckv_bf[:, kt, :],
                            start=(ci == 0 and kt == 0),
                            stop=False,
                        )
                # new-token chunk (4 cols)
                ps9 = sc_psum.tile([64, T], F32, tag="sc")
                for kt in range(ZKT):
                    nc.tensor.matmul(
                        ps9,
                        qaugT_bf[:, kt, :],
                        znewT_bf[:, kt, :],
                        start=(kt == 0),
                        stop=(kt == ZKT - 1),
                    )
                nc.scalar.activation(
                    out=exp_sb[:, S0:S],
                    in_=ps9,
                    func=mybir.ActivationFunctionType.Exp,
                    accum_out=sums[:, NCHUNKS: NCHUNKS + 1],
                )
                ptn = tp_psum.tile([T, 64], BF16, tag="tp")
                nc.tensor.transpose(ptn, exp_sb[:, S0:S], id_bf[0:64, 0:64])
                nc.vector.tensor_copy(out=attnT[0:T, S0 // 128, :], in_=ptn)
                nc.tensor.matmul(
                    pso,
                    attnT[0:T, S0 // 128, :],
                    ckvnew_bf,
                    start=False,
                    stop=True,
                )
                # softmax denominator, applied in latent space
                total = singles.tile([64, 1], F32)
                recip = singles.tile([64, 1], F32)
                nc.vector.reduce_sum(
                    out=total, in_=sums, axis=mybir.AxisListType.X
                )
                nc.vector.reciprocal(out=recip, in_=total)
                nc.vector.tensor_copy(out=o_lat, in_=pso)
                nc.vector.tensor_scalar_mul(o_lat, o_lat, recip)

            # w_up arrives behind the zt/ckv stream, well before stage 10
            nc.sync.dma_start(out=wup_sb, in_=d_wup)

            # o_latT [512, 64] in bf16
            olat_bf = singles.tile([64, RANK], BF16)
            nc.vector.tensor_copy(out=olat_bf, in_=o_lat)
            olatT = singles.tile([128, RANK // 128, 64], BF16)
            for rt in range(RANK // 128):
                pt = tp_psum.tile([128, 64], BF16, tag="tp")
                nc.tensor.transpose(
                    pt, olat_bf[:, rt * 128:(rt + 1) * 128], id_bf[0:64, 0:64]
                )
                nc.vector.tensor_copy(out=olatT[:, rt, :], in_=pt)

            # ---------- stages 10+11 -------------------------------------------
            aoT = singles.tile([128, H, T], BF16)
            out_stage = singles.tile([T, D_MODEL], F32)
            d_wout_r = d_wout.rearrange("(kt p) n -> p kt n", p=128)
            with tc.tile_pool(name="wout_pool", bufs=4) as wout_pool:
                # all four w_out quarter-DMAs issue up front (deep prefetch;
                # they reuse the zt/ckv pool space freed above)
                wout_tiles = []
                for g in range(4):
                    wt = wout_pool.tile([128, 4, D_MODEL], BF16, tag="wout_bf")
                    nc.sync.dma_start(out=wt, in_=d_wout_r[:, 4 * g: 4 * g + 4, :])
                    wout_tiles.append(wt)

                # stage 10: attn_outT [2048, 4] = blockdiag(w_up^T) @ o_latT
                with tc.tile_pool(name="ao_psum", bufs=2, space="PSUM") as ao_psum:
                    for h in range(H):
                        ps = ao_psum.tile([128, T], F32)
                        for rt in range(RANK // 128):
                            nc.tensor.matmul(
                                ps,
                                wup_sb[:, rt, h * D:(h + 1) * D],
                                olatT[:, rt, 4 * h: 4 * h + 4],
                                start=(rt == 0),
                                stop=(rt == RANK // 128 - 1),
                            )
                        nc.vector.tensor_copy(out=aoT[:, h, :], in_=ps)

                # stage 11: out = attn_out @ w_out [4, 2048]
                with tc.tile_pool(name="out_psum", bufs=1, space="PSUM") as out_psum:
                    pouts = [
                        out_psum.tile([T, 512], F32, name=f"pout{i}", tag=f"pout{i}")
                        for i in range(4)
                    ]
                    for kt in range(H * D // 128):  # 16
                        wt = wout_tiles[kt // 4]
                        for ci in range(4):
                            # D == 128: attn_outT k-tile kt is exactly head kt
                            nc.tensor.matmul(
                                pouts[ci],
                                aoT[:, kt, :],
                                wt[:, kt % 4, ci * 512:(ci + 1) * 512],
                                start=(kt == 0),
                                stop=(kt == H * D // 128 - 1),
                            )
                    for ci in range(4):
                        nc.vector.tensor_copy(
                            out=out_stage[:, ci * 512:(ci + 1) * 512], in_=pouts[ci]
                        )
                    nc.sync.dma_start(out=d_out, in_=out_stage)

    nc.compile()
    return nc


def _rope(nc, rot, raw, tmpA, tmpB, ctab_sb, stab_sb):
    """rot[t, h*64+j]      = raw1*cos - raw2*sin   (j in [0,32))
       rot[t, h*64+32+j]   = raw1*sin + raw2*cos
    using A = raw*ctab, B = raw*stab with half-replicated cos/sin tables."""
    nc.vector.tensor_mul(tmpA, raw, ctab_sb)
    nc.vector.tensor_mul(tmpB, raw, stab_sb)
    a3 = tmpA.rearrange("p (h r) -> p h r", h=H)
    b3 = tmpB.rearrange("p (h r) -> p h r", h=H)
    r3 = rot.rearrange("p (h r) -> p h r", h=H)
    nc.vector.tensor_sub(r3[:, :, 0:32], a3[:, :, 0:32], b3[:, :, 32:64])
    nc.vector.tensor_add(r3[:, :, 32:64], b3[:, :, 0:32], a3[:, :, 32:64])


def _prep_shared(w_q, w_q_rope, w_kv_down, w_kv_up, w_k_rope, w_out):
    scale = np.float32(1.0 / np.sqrt(D))
    w_in = np.concatenate(
        [w_q * scale, w_q_rope * scale, w_kv_down, w_k_rope], axis=1
    )
    # chunked: [7*128, 16, 512] with row p of chunk ci holding
    # w_in[kt*128+p, ci*512:(ci+1)*512] for kt = 0..15 (contiguous /partition)
    w_in_c = np.ascontiguousarray(
        w_in.reshape(16, 128, PROJ_N // 512, 512).transpose(2, 1, 0, 3)
    ).reshape((PROJ_N // 512) * 128, 16, 512).astype(NPBF)
    w_up_c = np.ascontiguousarray(
        w_kv_up.reshape(RANK // 128, 128, H * D).transpose(1, 0, 2)
    ).astype(NPBF)                                    # [128, 4, 2048]
    w_up_tc = np.ascontiguousarray(
        w_kv_up.reshape(RANK, H, D)[:, :, :C].transpose(2, 1, 0)
    ).reshape(C, H * RANK).astype(NPBF)
    w_out_c = np.ascontiguousarray(w_out).astype(NPBF)
    pos = S0 + np.arange(T)
    invf = 1.0 / (10000.0 ** (np.arange(R // 2) / (R // 2)))
    f = (pos[:, None] * invf[None, :]).astype(np.float32)
    cos32, sin32 = np.cos(f).astype(np.float32), np.sin(f).astype(np.float32)
    ctab = np.ascontiguousarray(
        np.tile(np.concatenate([cos32, cos32], 1)[:, None, :], (1, H, 1))
    ).reshape(T, H * R)
    stab = np.ascontiguousarray(
        np.tile(np.concatenate([sin32, sin32], 1)[:, None, :], (1, H, 1))
    ).reshape(T, H * R)
    return w_in_c, w_up_c, w_up_tc, w_out_c, ctab, stab


def _make_in_maps(inputs):
    x = np.asarray(inputs["x"], np.float32)
    cache_c_kv = np.asarray(inputs["cache_c_kv"], np.float32)
    cache_k_rope = np.asarray(inputs["cache_k_rope"], np.float32)
    w_in_c, w_up_c, w_up_tc, w_out_c, ctab, stab = _prep_shared(
        np.asarray(inputs["w_q"], np.float32),
        np.asarray(inputs["w_q_rope"], np.float32),
        np.asarray(inputs["w_kv_down"], np.float32),
        np.asarray(inputs["w_kv_up"], np.float32),
        np.asarray(inputs["w_k_rope"], np.float32),
        np.asarray(inputs["w_out"], np.float32))

    in_maps = []
    for b in range(B):
        zt = np.concatenate(
            [cache_c_kv[b].T,
             cache_k_rope[b].transpose(1, 2, 0).reshape(H * R, S0)],
            axis=0,
        )  # [1536, 4096]
        # chunked: [8*128, 12, 512], chunk ci row p = z_t[kt*128+p, ci*512:...]
        zt_c = np.ascontiguousarray(
            zt.reshape(ZKT, 128, NCHUNKS, SCHUNK).transpose(2, 1, 0, 3)
        ).reshape(NCHUNKS * 128, ZKT, SCHUNK).astype(NPBF)
        # chunked row-major c_kv: [8*128, 4, 512], s = kg*512 + kt*128 + p
        ckv_c = np.ascontiguousarray(
            cache_c_kv[b].reshape(NCHUNKS, 4, 128, RANK).transpose(0, 2, 1, 3)
        ).reshape(NCHUNKS * 128, 4, RANK).astype(NPBF)
        xt_c = np.ascontiguousarray(
            x[b].T.reshape(D_MODEL // 128, 128, T).transpose(1, 0, 2)
        ).astype(NPBF)  # [128, 16, 4]
        in_maps.append({
            "xT": xt_c,
            "w_in": w_in_c,
            "w_up": w_up_c,
            "w_up_tc": w_up_tc,
            "w_out": w_out_c,
            "z_t": zt_c,
            "c_kv": ckv_c,
            "ctab": ctab,
            "stab": stab,
        })
    return in_maps


def kernel(x, cache_c_kv, cache_k_rope, w_q, w_q_rope, w_kv_down, w_kv_up,
           w_k_rope, w_out):
    global _BUILT, _LAST_RESULTS
    if _BUILT is None:
        _BUILT = build_bass()
    nc = _BUILT

    in_maps = _make_in_maps(dict(
        x=x, cache_c_kv=cache_c_kv, cache_k_rope=cache_k_rope, w_q=w_q,
        w_q_rope=w_q_rope, w_kv_down=w_kv_down, w_kv_up=w_kv_up,
        w_k_rope=w_k_rope, w_out=w_out))

    res = bass_utils.run_bass_kernel_spmd(nc, in_maps, core_ids=list(range(B)))
    _LAST_RESULTS = res
    return np.stack([res.results[b]["out"] for b in range(B)]).astype(np.float32)
